# revision 24
# baseline (speedup 1.0000x reference)
"""GAT (graph attention) kernel for Trainium2, 8-core row-parallel SPMD.

Math (matches the reference exactly):
    h   = einsum('nm,hmf->hnf', x, W)                  # [H, N, F]
    ci  = h @ wi ; cj = h @ wj                         # [H, N]
    e   = exp(leaky_relu(ci[:,None] + cj[None,:], 0.2))
    adj = (graph > 0).T                                # mask[i, j] = graph[j, i] > 0
    att = softmax where adj, else 0
    y   = att @ h + x @ Wr + bias

Factoring used on device (no exp over the NxN plane):
    exp(lrelu(t)) = max(exp(t), exp(0.2 t)),  t = ci + cj
    with v=exp(cj), v'=exp(0.2 cj), r=exp(-0.8 ci):  e/u = max(v_j, v'_j r_i)
    (u=exp(ci) cancels in the softmax).

Engine split per j-tile (the [j,i] score plane is the whole cost):
  - all 4 head planes are produced by a CUSTOM fused DVE instruction
    (GAT_SCORE_MASK_ANT below):  wt = max(r*v', v) * adj  in one pass, with
    per-partition scalars v'=exp(0.2 cj), v=exp(cj), tensor operands
    r_i (partition-broadcast) and the adjacency tile. The op ships a
    hand-written 2X_1PORT uop program (HW-validated): the stock custom-DVE
    path has no perf-mode table slots, so without it each plane would cost
    2x more DVE time than the tensor_scalar+tensor_tensor pair it replaces.
  - aggregation: psum[f|den, i] += H[j, f|1]^T @ wt[j, i]  (f16, 8 matmuls;
    the ones-column of H makes the softmax denominator fall out of the
    same matmul).

h / ci / cj come from one fp8 DoubleRow matmul per j-tile (contraction
256 = 2 k-tiles of 128; the stationary x.T slice satisfies the 16B pair-
stride rule, and fp8 is fine for the attention path since softmax
normalizes and the tolerance budget is loose). The residual x @ Wr runs
in bf16 off a per-core x.T slice, which preserves output accuracy.

Sharding: core c owns output rows [c*1024, (c+1)*1024). Host sends x.T
replicated (fp8 [128,2,N] k-tile layout + bf16/fp8 per-core column slices)
and the core's graph column-slice as fp8 {0,1} (natural [j, i] layout =
the transposed mask the reference uses). The mask is DMA'd at 1 byte/edge
(half the HBM traffic of f16) and upconverted to f16 per tile on the
otherwise-idle Activation engine before the fused DVE op consumes it.
"""

import numpy as np

import concourse.bass as bass
import concourse.tile as tile
from concourse import bacc, dve_ops, mybir
from concourse.bass_utils import run_bass_kernel_spmd
from concourse.dve_spec import Spec, lower, Src0, Src1, C0, C1, maxx
from concourse.dve_table_gen import dve_ver_for
from concourse.dve_uop import (AluInp, AluOp, DelayInp, DveOpSpec, InpSel,
                               OutPath, OutSel, Trigger, UopConfig, UopDpConfig)


# --- custom fused DVE op: out = max(in0*s0, s1)*in1, with a hand-written
# 2X_1PORT uop program (validated on hardware against numpy; the stock
# custom-DVE path ships no perf-mode programs, so this is what lets one
# instruction per head-plane replace tensor_scalar + tensor_tensor at the
# DVE's two-tensor-op throughput ceiling).
_GSM_NAME = "GAT_SCORE_MASK_ANT"
_GSM_SPEC = Spec(
    body=maxx(Src0 * C0, C1) * Src1,
    reference=lambda in0, in1, s0, s1, imm2: (
        np.maximum(in0.astype(np.float32) * s0, s1) * in1),
)


def _gsm_dp(op, src0, src1, lane0=None):
    d = [DelayInp.PREV_DELAY] * 6 + [DelayInp.PREV_ALU_OUT]
    if lane0 is not None:
        d[0] = lane0
    return UopDpConfig(op=op, alu_src0=src0, alu_src1=src1, delay=d,
                       alu_out_enable=1, delay_enable=[1, 1, 1, 1, 1, 1, 0])


def _gsm_2x():
    P = AluInp
    stages = [
        # elem 0: max(src0*c0, c1)*src1  (lanes: 0=src0 1=c0 2=c1 3=src1
        #                                        4=src0_hi 5=src1_hi)
        _gsm_dp(AluOp.MULTIPLY, P.PREV_DELAY_0, P.PREV_DELAY_1),
        _gsm_dp(AluOp.MAX, P.PREV_ALU_OUT, P.PREV_DELAY_2),
        _gsm_dp(AluOp.MULTIPLY, P.PREV_ALU_OUT, P.PREV_DELAY_3),
        # elem 1 (HI): same, capturing the elem-0 result into freed lane 0
        _gsm_dp(AluOp.MULTIPLY, P.PREV_DELAY_4, P.PREV_DELAY_1,
                lane0=DelayInp.PREV_ALU_OUT),
        _gsm_dp(AluOp.MAX, P.PREV_ALU_OUT, P.PREV_DELAY_2),
        _gsm_dp(AluOp.MULTIPLY, P.PREV_ALU_OUT, P.PREV_DELAY_5),
        _gsm_dp(AluOp.BYPASS, P.PREV_ALU_OUT, P.PREV_ALU_OUT),
        _gsm_dp(AluOp.BYPASS, P.PREV_ALU_OUT, P.PREV_ALU_OUT),
    ]
    return UopConfig(
        inp=[InpSel.ZERO, InpSel.SRC_0, InpSel.CONST_0, InpSel.CONST_1,
             InpSel.SRC_1, InpSel.SRC_0_HI, InpSel.SRC_1_HI, InpSel.ZERO],
        inp_enable=[0, 1, 1, 1, 1, 1, 1, 0],
        out={OutPath.WR0_LO: OutSel.DELAY_0, OutPath.WR0_HI: OutSel.ALU_OUT,
             OutPath.WR1_LO: OutSel.ALU_OUT, OutPath.WR1_HI: OutSel.ALU_OUT},
        out_enable={OutPath.WR0_LO: 1, OutPath.WR0_HI: 1,
                    OutPath.WR1_LO: 0, OutPath.WR1_HI: 0},
        require_inp0=1, require_inp1=1,
        trigger=(Trigger.SRC_TENSOR_DONE, Trigger.NONE, Trigger.NONE),
        next_uop=(0, 0, 0),
        datapath_config=stages,
    )


def _register_gsm():
    if _GSM_NAME in dve_ops._SUB_OPCODE_FOR_NAME:
        return next(o for o in dve_ops.OPS if o.name == _GSM_NAME)
    ver = dve_ver_for("TRN2")
    row = 1 + len(dve_ops.OPS)
    op = dve_ops.DveOp(_GSM_NAME, _GSM_SPEC, subdim=False, uops_sha={})
    dve_ops.OPS.append(op)
    dve_ops.CUSTOM_DVE_SPECS[_GSM_NAME] = _GSM_SPEC
    dve_ops._SUB_OPCODE_FOR_NAME[_GSM_NAME] = row
    cspec = DveOpSpec(name=_GSM_NAME, opcode=row, uops=lower(_GSM_SPEC, ver=ver),
                      uops_2x=[_gsm_2x()], perf_max=2, rd1_en=True)
    cspec.validate(ver)
    dve_ops._COMPILE_CACHE[(_GSM_NAME, ver)] = cspec
    return op


_GSM_OP = _register_gsm()

N = 8192
IN_F = 256
HEADS = 4
HF = 64
OUT_F = HEADS * HF
NCORES = 8
ROWS = N // NCORES          # 1024 output rows per core
NJT = N // 128              # 64 j tiles of 128
MT = IN_F // 128            # 2 k-tiles of the input-feature contraction
ICH = ROWS // 512           # 2 moving-operand chunks of 512
HC = HF + 2                 # per-head columns in the h matmul: 64 h + ones + cj
ACT_HEADS = (2, 3)          # planes produced on Activation (relu form)
DVE_HEAD = 0                # plane produced on DVE (max form)
POOL_HEAD = 1               # plane produced on Pool/gpsimd (max form)

F32 = mybir.dt.float32
F16 = mybir.dt.float16
BF16 = mybir.dt.bfloat16
F8 = mybir.dt.float8e4
ALU = mybir.AluOpType
AF = mybir.ActivationFunctionType
DR = mybir.MatmulPerfMode.DoubleRow


def _build_program(loop_reps=None):
    nc = bacc.Bacc("TRN2", target_bir_lowering=False, debug=False)

    x8_d = nc.dram_tensor("xt8", [128, MT, N], F8, kind="ExternalInput")
    xi8_d = nc.dram_tensor("xit8", [128, MT, ROWS], F8, kind="ExternalInput")
    xi_d = nc.dram_tensor("xit", [IN_F, ROWS], BF16, kind="ExternalInput")
    g8_d = nc.dram_tensor("gcol8", [N, ROWS], F8, kind="ExternalInput")
    w_d = nc.dram_tensor("weight", [HEADS, IN_F, HF], F32, kind="ExternalInput")
    wi_d = nc.dram_tensor("weight_i", [HEADS, HF, 1], F32, kind="ExternalInput")
    wj_d = nc.dram_tensor("weight_j", [HEADS, HF, 1], F32, kind="ExternalInput")
    wr_d = nc.dram_tensor("weight_r", [IN_F, OUT_F], F32, kind="ExternalInput")
    b_d = nc.dram_tensor("bias", [OUT_F], F32, kind="ExternalInput")
    id_d = nc.dram_tensor("ident", [128, 128], F32, kind="ExternalInput")
    y_d = nc.dram_tensor("y", [ROWS, OUT_F], F32, kind="ExternalOutput")
    dts = (x8_d, xi8_d, xi_d, g8_d, w_d, wi_d, wj_d, wr_d, b_d, id_d, y_d)

    with tile.TileContext(nc) as tc:
        if loop_reps is None:
            _gat_body(tc, *dts)
        else:
            with tc.For_i(0, loop_reps, 1):
                _gat_body(tc, *dts)
    nc.compile()
    return nc


def _gat_body(tc, *dts):
    nc = tc.nc
    with tc.tile_pool(name="consts", bufs=1) as consts, \
         tc.tile_pool(name="persist", bufs=1) as persist:
        _gat_inner(tc, nc, consts, persist, *dts)


def _gat_inner(tc, nc, consts, persist,
               x8_d, xi8_d, xi_d, g8_d, w_d, wi_d, wj_d, wr_d, b_d, id_d, y_d):
    ident = consts.tile([128, 128], F32, name="ident", tag="ident")
    nc.sync.dma_start(ident[:], id_d[:, :])
    ones1 = consts.tile([1, 128], BF16, name="ones1", tag="ones1")
    nc.vector.memset(ones1[:], 1.0)

    xT8 = persist.tile([128, MT, N], F8, name="xT8", tag="xT8")
    xiT8 = persist.tile([128, MT, ROWS], F8, name="xiT8", tag="xiT8")
    xiT = [persist.tile([128, ROWS], BF16, name=f"xiT{mt}", tag=f"xiT{mt}") for mt in range(MT)]
    Hb = persist.tile([128, NJT, HEADS, HC], F16, name="Hb", tag="Hb")
    Rb = [persist.tile([128, ROWS], F16, name=f"Rb{h}", tag=f"Rb{h}") for h in range(HEADS)]
    vs = [persist.tile([128, NJT], F32, name=f"v{h}", tag=f"v{h}") for h in range(HEADS)]
    vps = [persist.tile([128, NJT], F32, name=f"vp{h}", tag=f"vp{h}") for h in range(HEADS)]

    with tc.tile_pool(name="ph0", bufs=3) as ph0, \
         tc.tile_pool(name="ph0ps", bufs=3, space="PSUM") as ph0ps:
        # --- x loads ---
        for q in range(8):
            sl = slice(q * (N // 8), (q + 1) * (N // 8))
            eng = nc.sync if q % 2 == 0 else nc.scalar
            eng.dma_start(xT8[:, :, sl], x8_d[:, :, sl])
        nc.sync.dma_start(xiT8[:], xi8_d[:])
        for mt in range(MT):
            nc.sync.dma_start(xiT[mt][:], xi_d[mt * 128:(mt + 1) * 128, :])

        # --- small weights ---
        wrf = [ph0.tile([128, OUT_F], F32, name=f"wrf{mt}", tag="wrf", bufs=2) for mt in range(MT)]
        wr_sb = [consts.tile([128, OUT_F], BF16, name=f"wr{mt}", tag=f"wr{mt}") for mt in range(MT)]
        for mt in range(MT):
            nc.sync.dma_start(wrf[mt][:], wr_d[mt * 128:(mt + 1) * 128, :])
            nc.vector.tensor_copy(wr_sb[mt][:], wrf[mt][:])
        biasf = ph0.tile([1, OUT_F], F32, name="biasf", tag="biasf", bufs=1)
        nc.sync.dma_start(biasf[:], b_d.ap().rearrange("(a b) -> a b", a=1))
        bias_sb = consts.tile([1, OUT_F], BF16, name="bias", tag="bias")
        nc.vector.tensor_copy(bias_sb[:], biasf[:])
        wij_sb = []
        for h in range(HEADS):
            t = consts.tile([HF, 2], F32, name=f"wij{h}", tag=f"wij{h}")
            nc.sync.dma_start(t[:, 0:1], wi_d[h])
            nc.sync.dma_start(t[:, 1:2], wj_d[h])
            wij_sb.append(t)

        # rhsall8[p, kt, h*HC + c]: per-head [W_h(64) | A_h | B_h] moving
        # operand of the h matmul (fp8, k = kt*128 + p), A = W @ wi, B = W @ wj.
        rhsall8 = consts.tile([128, MT, HEADS * HC], F8, name="rhsall8", tag="rhsall8")
        wtmp = [consts.tile([128, HEADS * HF], F32, name=f"wtmp{mt}", tag=f"wtmp{mt}") for mt in range(MT)]
        for mt in range(MT):
            for h in range(HEADS):
                nc.sync.dma_start(wtmp[mt][:, h * HF:(h + 1) * HF],
                                  w_d[h, mt * 128:(mt + 1) * 128, :])
                nc.vector.tensor_copy(rhsall8[:, mt, h * HC:h * HC + HF],
                                      wtmp[mt][:, h * HF:(h + 1) * HF])

        # W_h^T (for A/B columns): transpose the [m, f] weight slices.
        whT = [consts.tile([HF, IN_F], F32, name=f"whT{h}", tag=f"whT{h}") for h in range(HEADS)]
        for h in range(HEADS):
            for mt in range(MT):
                ps = ph0ps.tile([HF, 128], F32, name="wtps", tag="wtps", bufs=1)
                nc.tensor.transpose(ps[:], wtmp[mt][:, h * HF:(h + 1) * HF], ident[:])
                nc.vector.tensor_copy(whT[h][:, mt * 128:(mt + 1) * 128], ps[:])
        for h in range(HEADS):
            for mt in range(MT):
                psab = ph0ps.tile([128, 2], F32, name="abps", tag="abps", bufs=1)
                nc.tensor.matmul(psab[:], whT[h][:, mt * 128:(mt + 1) * 128],
                                 wij_sb[h][:], start=True, stop=True)
                nc.vector.tensor_copy(rhsall8[:, mt, h * HC + HF:h * HC + HF + 2], psab[:])

        # --- r_i = exp(-0.8 ci) for our rows (early: unblocks main-loop lead) ---
        for h in range(HEADS):
            rrow = persist.tile([1, ROWS], F16, name=f"rrow{h}", tag=f"rrow{h}")
            for ch in range(ICH):
                psci = ph0ps.tile([2, 512], F32, name="psci", tag="psci", bufs=1)
                for mt in range(MT):
                    nc.tensor.matmul(psci[:], rhsall8[:, mt, h * HC + HF:h * HC + HF + 2],
                                     xiT8[:, mt, ch * 512:(ch + 1) * 512],
                                     start=(mt == 0), stop=(mt == MT - 1))
                nc.scalar.activation(rrow[0:1, ch * 512:(ch + 1) * 512],
                                     psci[0:1, :], AF.Exp, scale=-0.8)
            nc.gpsimd.partition_broadcast(Rb[h][:], rrow[:])

        # --- h / ci / cj for all N rows (fp8 DoubleRow: contraction 2x128) ---
        for jt in range(NJT):
            psh = ph0ps.tile([128, HEADS * HC], F32, name="psh", tag="psh", bufs=5)
            nc.tensor.matmul(psh[:], xT8[:, :, jt * 128:(jt + 1) * 128],
                             rhsall8[:], start=True, stop=True, perf_mode=DR)
            hdst = Hb[:, jt].rearrange("p a b -> p (a b)")
            if jt % 2 == 0:
                nc.scalar.copy(hdst, psh[:])
            else:
                nc.vector.tensor_copy(hdst, psh[:])

    # --- per-head j-row scalars from the cj column of Hb ---
    for h in range(HEADS):
        cjap = Hb[:, :, h, HF + 1]
        nc.scalar.activation(vs[h][:], cjap, AF.Exp)
        nc.scalar.activation(vps[h][:], cjap, AF.Exp, scale=0.2)
        # ones column for the softmax denominator (overwrites the unused slot)
        nc.scalar.activation(Hb[:, :, h, HF], cjap, AF.Copy, bias=1.0, scale=0.0)


    # --- main loop: scores + aggregation ---
    with tc.tile_pool(name="psy", bufs=HEADS * ICH, space="PSUM") as psy_pool:
        psy = [[psy_pool.tile([HF + 1, 512], F32, name="psy", tag="psy") for _ in range(ICH)]
               for _ in range(HEADS)]
        with tc.tile_pool(name="mainl", bufs=3) as ml:
            for jb in range(NJT):
                g8t = ml.tile([128, ROWS], F8, name="g8t", tag="g8t", bufs=3)
                nc.sync.dma_start(g8t[:], g8_d[jb * 128:(jb + 1) * 128, :])
                adj = ml.tile([128, ROWS], F16, name="adj", tag="adj", bufs=3)
                nc.scalar.copy(adj[:], g8t[:])
                wt4 = ml.tile([128, HEADS, ROWS], F16, name="wt4", tag="wt4", bufs=3)
                for h in range(HEADS):
                    ins = nc.vector._custom_dve(
                        _GSM_OP, out=wt4[:, h, :], in0=Rb[h][:], in1=adj[:],
                        s0=vps[h][:, jb:jb + 1], s1=vs[h][:, jb:jb + 1])
                    ins.ins.perf_max = 2
                last = jb == NJT - 1
                for h in range(HEADS):
                    for ch in range(ICH):
                        nc.tensor.matmul(psy[h][ch][:], Hb[:, jb, h, 0:HF + 1],
                                         wt4[:, h, ch * 512:(ch + 1) * 512],
                                         start=(jb == 0), stop=last)

        # copy numerators/denominator out of PSUM (releases psy banks)
        ysb = [[persist.tile([HF + 1, 512], F32, name=f"ysb{h}_{ch}", tag=f"ysb{h}_{ch}") for ch in range(ICH)]
               for h in range(HEADS)]
        for h in range(HEADS):
            for ch in range(ICH):
                if (h + ch) % 2 == 0:
                    nc.scalar.copy(ysb[h][ch][:], psy[h][ch][:])
                else:
                    nc.vector.tensor_copy(ysb[h][ch][:], psy[h][ch][:])

    # --- output: transpose to [i, f], divide by denominator, add residual ---
    with tc.tile_pool(name="outps", bufs=3, space="PSUM") as outps, \
         tc.tile_pool(name="outsb", bufs=3) as outsb:
        for it in range(ROWS // 128):
            ch, off = divmod(it * 128, 512)
            pso = outps.tile([128, HEADS, HF + 1], F32, name="pso", tag="pso")
            for h in range(HEADS):
                nc.tensor.transpose(pso[:, h, :], ysb[h][ch][:, off:off + 128],
                                    ident[0:HF + 1, 0:HF + 1])
            rden = outsb.tile([128, HEADS], F32, name="rden", tag="rden")
            nc.vector.reciprocal(rden[:], pso[:, :, HF])
            yatt = outsb.tile([128, OUT_F], F32, name="yatt", tag="yatt")
            for h in range(HEADS):
                nc.scalar.activation(yatt[:, h * HF:(h + 1) * HF], pso[:, h, 0:HF],
                                     AF.Copy, scale=rden[:, h:h + 1])
            psr = outps.tile([128, OUT_F], F32, name="psr", tag="psr")
            for mt in range(MT):
                nc.tensor.matmul(psr[:], xiT[mt][:, it * 128:(it + 1) * 128],
                                 wr_sb[mt][:], start=(mt == 0), stop=False)
            nc.tensor.matmul(psr[:], ones1[:], bias_sb[:], start=False, stop=True)
            out_t = outsb.tile([128, OUT_F], F32, name="outt", tag="outt")
            nc.vector.tensor_tensor(out_t[:], yatt[:], psr[:], ALU.add)
            nc.sync.dma_start(y_d[it * 128:(it + 1) * 128, :], out_t[:])


_NC_CACHE = {}


def _get_program(loop_reps=None):
    if loop_reps not in _NC_CACHE:
        _NC_CACHE[loop_reps] = _build_program(loop_reps)
    return _NC_CACHE[loop_reps]


def _make_in_maps(x, graph, weight, weight_i, weight_j, weight_r, bias):
    import ml_dtypes
    f8 = mybir.dt.np(F8)
    x = np.asarray(x, dtype=np.float32)
    xt = np.ascontiguousarray(x.T)                      # [IN_F, N] f32
    xt8 = np.ascontiguousarray(                         # [128, MT, N] fp8
        xt.reshape(MT, 128, N).transpose(1, 0, 2).astype(f8))
    xt_bf = xt.astype(ml_dtypes.bfloat16)
    g8 = np.asarray(graph, dtype=np.float32).astype(f8)
    maps = []
    for c in range(NCORES):
        i0 = c * ROWS
        maps.append({
            "xt8": xt8,
            "xit8": np.ascontiguousarray(xt8[:, :, i0:i0 + ROWS]),
            "xit": np.ascontiguousarray(xt_bf[:, i0:i0 + ROWS]),
            "gcol8": np.ascontiguousarray(g8[:, i0:i0 + ROWS]),
            "weight": np.ascontiguousarray(weight, dtype=np.float32),
            "weight_i": np.ascontiguousarray(weight_i, dtype=np.float32),
            "weight_j": np.ascontiguousarray(weight_j, dtype=np.float32),
            "weight_r": np.ascontiguousarray(weight_r, dtype=np.float32),
            "bias": np.ascontiguousarray(bias, dtype=np.float32),
            "ident": np.eye(128, dtype=np.float32),
        })
    return maps


def _run(in_maps, loop_reps=None):
    nc = _get_program(loop_reps)
    res = run_bass_kernel_spmd(nc, in_maps, list(range(NCORES)))
    return np.concatenate([res.results[c]["y"] for c in range(NCORES)], axis=0)


def kernel(x, graph, weight, weight_i, weight_j, weight_r, bias):
    in_maps = _make_in_maps(x, graph, weight, weight_i, weight_j, weight_r, bias)
    return _run(in_maps).astype(np.float32)


# revision 26
# speedup vs baseline: 1.1455x; 1.1455x over previous
"""GAT (graph attention) kernel for Trainium2, 8-core row-parallel SPMD.

Math (matches the reference exactly):
    h   = einsum('nm,hmf->hnf', x, W)                  # [H, N, F]
    ci  = h @ wi ; cj = h @ wj                         # [H, N]
    e   = exp(leaky_relu(ci[:,None] + cj[None,:], 0.2))
    adj = (graph > 0).T                                # mask[i, j] = graph[j, i] > 0
    att = softmax where adj, else 0
    y   = att @ h + x @ Wr + bias

Factoring used on device (no exp over the NxN plane):
    exp(lrelu(t)) = max(exp(t), exp(0.2 t)),  t = ci + cj
    with v=exp(cj), v'=exp(0.2 cj), r=exp(-0.8 ci):  e/u = max(v_j, v'_j r_i)
    (u=exp(ci) cancels in the softmax).

Engine split per j-tile (the [j,i] score plane is the whole cost):
  - all 4 head planes are produced by a CUSTOM fused DVE instruction
    (GAT_SCORE_MASK_ANT below):  wt = max(r*v', v) * adj  in one pass, with
    per-partition scalars v'=exp(0.2 cj), v=exp(cj), tensor operands
    r_i (partition-broadcast) and the adjacency tile. The op ships a
    hand-written 2X_1PORT uop program (HW-validated): the stock custom-DVE
    path has no perf-mode table slots, so without it each plane would cost
    2x more DVE time than the tensor_scalar+tensor_tensor pair it replaces.
  - aggregation: psum[f|den, i] += H[j, f|1]^T @ wt[j, i]  (f16, 8 matmuls;
    the ones-column of H makes the softmax denominator fall out of the
    same matmul).

h / ci / cj come from one fp8 DoubleRow matmul per j-tile (contraction
256 = 2 k-tiles of 128; the stationary x.T slice satisfies the 16B pair-
stride rule, and fp8 is fine for the attention path since softmax
normalizes and the tolerance budget is loose). The residual x @ Wr runs
in bf16 off a per-core x.T slice, which preserves output accuracy.

Sharding: core c owns output rows [c*1024, (c+1)*1024). Host sends x.T
replicated (fp8 [128,2,N] k-tile layout + bf16/fp8 per-core column slices)
and the core's graph column-slice as fp8 {0,1} (natural [j, i] layout =
the transposed mask the reference uses). The mask is DMA'd at 1 byte/edge
(half the HBM traffic of f16) and upconverted to f16 per tile on the
otherwise-idle Activation engine before the fused DVE op consumes it.
"""

import numpy as np

import concourse.bass as bass
import concourse.tile as tile
from concourse import bacc, dve_ops, mybir
from concourse.bass_utils import run_bass_kernel_spmd
from concourse.dve_spec import Spec, lower, Src0, Src1, C0, C1, maxx
from concourse.dve_table_gen import dve_ver_for
from concourse.dve_uop import (AluInp, AluOp, DelayInp, DveOpSpec, InpSel,
                               OutPath, OutSel, Trigger, UopConfig, UopDpConfig)


# --- custom fused DVE op: out = max(in0*s0, s1)*in1, with a hand-written
# 2X_1PORT uop program (validated on hardware against numpy; the stock
# custom-DVE path ships no perf-mode programs, so this is what lets one
# instruction per head-plane replace tensor_scalar + tensor_tensor at the
# DVE's two-tensor-op throughput ceiling).
_GSM_NAME = "GAT_SCORE_MASK_ANT"
_GSM_SPEC = Spec(
    body=maxx(Src0 * C0, C1) * Src1,
    reference=lambda in0, in1, s0, s1, imm2: (
        np.maximum(in0.astype(np.float32) * s0, s1) * in1),
)


def _gsm_dp(op, src0, src1, lane0=None):
    d = [DelayInp.PREV_DELAY] * 6 + [DelayInp.PREV_ALU_OUT]
    if lane0 is not None:
        d[0] = lane0
    return UopDpConfig(op=op, alu_src0=src0, alu_src1=src1, delay=d,
                       alu_out_enable=1, delay_enable=[1, 1, 1, 1, 1, 1, 0])


def _gsm_2x():
    P = AluInp
    stages = [
        # elem 0: max(src0*c0, c1)*src1  (lanes: 0=src0 1=c0 2=c1 3=src1
        #                                        4=src0_hi 5=src1_hi)
        _gsm_dp(AluOp.MULTIPLY, P.PREV_DELAY_0, P.PREV_DELAY_1),
        _gsm_dp(AluOp.MAX, P.PREV_ALU_OUT, P.PREV_DELAY_2),
        _gsm_dp(AluOp.MULTIPLY, P.PREV_ALU_OUT, P.PREV_DELAY_3),
        # elem 1 (HI): same, capturing the elem-0 result into freed lane 0
        _gsm_dp(AluOp.MULTIPLY, P.PREV_DELAY_4, P.PREV_DELAY_1,
                lane0=DelayInp.PREV_ALU_OUT),
        _gsm_dp(AluOp.MAX, P.PREV_ALU_OUT, P.PREV_DELAY_2),
        _gsm_dp(AluOp.MULTIPLY, P.PREV_ALU_OUT, P.PREV_DELAY_5),
        _gsm_dp(AluOp.BYPASS, P.PREV_ALU_OUT, P.PREV_ALU_OUT),
        _gsm_dp(AluOp.BYPASS, P.PREV_ALU_OUT, P.PREV_ALU_OUT),
    ]
    return UopConfig(
        inp=[InpSel.ZERO, InpSel.SRC_0, InpSel.CONST_0, InpSel.CONST_1,
             InpSel.SRC_1, InpSel.SRC_0_HI, InpSel.SRC_1_HI, InpSel.ZERO],
        inp_enable=[0, 1, 1, 1, 1, 1, 1, 0],
        out={OutPath.WR0_LO: OutSel.DELAY_0, OutPath.WR0_HI: OutSel.ALU_OUT,
             OutPath.WR1_LO: OutSel.ALU_OUT, OutPath.WR1_HI: OutSel.ALU_OUT},
        out_enable={OutPath.WR0_LO: 1, OutPath.WR0_HI: 1,
                    OutPath.WR1_LO: 0, OutPath.WR1_HI: 0},
        require_inp0=1, require_inp1=1,
        trigger=(Trigger.SRC_TENSOR_DONE, Trigger.NONE, Trigger.NONE),
        next_uop=(0, 0, 0),
        datapath_config=stages,
    )


def _register_gsm():
    if _GSM_NAME in dve_ops._SUB_OPCODE_FOR_NAME:
        return next(o for o in dve_ops.OPS if o.name == _GSM_NAME)
    ver = dve_ver_for("TRN2")
    row = 1 + len(dve_ops.OPS)
    op = dve_ops.DveOp(_GSM_NAME, _GSM_SPEC, subdim=False, uops_sha={})
    dve_ops.OPS.append(op)
    dve_ops.CUSTOM_DVE_SPECS[_GSM_NAME] = _GSM_SPEC
    dve_ops._SUB_OPCODE_FOR_NAME[_GSM_NAME] = row
    cspec = DveOpSpec(name=_GSM_NAME, opcode=row, uops=lower(_GSM_SPEC, ver=ver),
                      uops_2x=[_gsm_2x()], perf_max=2, rd1_en=True)
    cspec.validate(ver)
    dve_ops._COMPILE_CACHE[(_GSM_NAME, ver)] = cspec
    return op


_GSM_OP = _register_gsm()

N = 8192
IN_F = 256
HEADS = 4
HF = 64
OUT_F = HEADS * HF
NCORES = 8
ROWS = N // NCORES          # 1024 output rows per core
NJT = N // 128              # 64 j tiles of 128
MT = IN_F // 128            # 2 k-tiles of the input-feature contraction
ICH = ROWS // 512           # 2 moving-operand chunks of 512
HC = HF + 2                 # per-head columns in the h matmul: 64 h + ones + cj
ACT_HEADS = (2, 3)          # planes produced on Activation (relu form)
DVE_HEAD = 0                # plane produced on DVE (max form)
POOL_HEAD = 1               # plane produced on Pool/gpsimd (max form)

F32 = mybir.dt.float32
F16 = mybir.dt.float16
BF16 = mybir.dt.bfloat16
F8 = mybir.dt.float8e4
ALU = mybir.AluOpType
AF = mybir.ActivationFunctionType
DR = mybir.MatmulPerfMode.DoubleRow


def _build_program(loop_reps=None):
    nc = bacc.Bacc("TRN2", target_bir_lowering=False, debug=False)

    x8_d = nc.dram_tensor("xt8", [128, MT, N], F8, kind="ExternalInput")
    xi8_d = nc.dram_tensor("xit8", [128, MT, ROWS], F8, kind="ExternalInput")
    xi_d = nc.dram_tensor("xit", [IN_F, ROWS], BF16, kind="ExternalInput")
    g8_d = nc.dram_tensor("gcol8", [N, ROWS], F8, kind="ExternalInput")
    w_d = nc.dram_tensor("weight", [HEADS, IN_F, HF], F32, kind="ExternalInput")
    wi_d = nc.dram_tensor("weight_i", [HEADS, HF, 1], F32, kind="ExternalInput")
    wj_d = nc.dram_tensor("weight_j", [HEADS, HF, 1], F32, kind="ExternalInput")
    wr_d = nc.dram_tensor("weight_r", [IN_F, OUT_F], F32, kind="ExternalInput")
    b_d = nc.dram_tensor("bias", [OUT_F], F32, kind="ExternalInput")
    id_d = nc.dram_tensor("ident", [128, 128], F32, kind="ExternalInput")
    y_d = nc.dram_tensor("y", [ROWS, OUT_F], F32, kind="ExternalOutput")
    dts = (x8_d, xi8_d, xi_d, g8_d, w_d, wi_d, wj_d, wr_d, b_d, id_d, y_d)

    with tile.TileContext(nc) as tc:
        if loop_reps is None:
            _gat_body(tc, *dts)
        else:
            with tc.For_i(0, loop_reps, 1):
                _gat_body(tc, *dts)
    nc.compile()
    return nc


def _gat_body(tc, *dts):
    nc = tc.nc
    with tc.tile_pool(name="consts", bufs=1) as consts, \
         tc.tile_pool(name="persist", bufs=1) as persist:
        _gat_inner(tc, nc, consts, persist, *dts)


def _gat_inner(tc, nc, consts, persist,
               x8_d, xi8_d, xi_d, g8_d, w_d, wi_d, wj_d, wr_d, b_d, id_d, y_d):
    ident = consts.tile([128, 128], F32, name="ident", tag="ident")
    nc.sync.dma_start(ident[:], id_d[:, :])
    ones1 = consts.tile([1, 128], BF16, name="ones1", tag="ones1")
    nc.vector.memset(ones1[:], 1.0)

    xT8 = persist.tile([128, MT, N], F8, name="xT8", tag="xT8")
    xiT8 = persist.tile([128, MT, ROWS], F8, name="xiT8", tag="xiT8")
    xiT = [persist.tile([128, ROWS], BF16, name=f"xiT{mt}", tag=f"xiT{mt}") for mt in range(MT)]
    Hb = persist.tile([128, NJT, HEADS, HC], F16, name="Hb", tag="Hb")
    Rb = [persist.tile([128, ROWS], F16, name=f"Rb{h}", tag=f"Rb{h}") for h in range(HEADS)]
    vs = [persist.tile([128, NJT], F32, name=f"v{h}", tag=f"v{h}") for h in range(HEADS)]
    vps = [persist.tile([128, NJT], F32, name=f"vp{h}", tag=f"vp{h}") for h in range(HEADS)]

    with tc.tile_pool(name="ph0", bufs=3) as ph0, \
         tc.tile_pool(name="ph0ps", bufs=3, space="PSUM") as ph0ps:
        # --- x loads ---
        for q in range(8):
            sl = slice(q * (N // 8), (q + 1) * (N // 8))
            eng = nc.sync if q % 2 == 0 else nc.scalar
            eng.dma_start(xT8[:, :, sl], x8_d[:, :, sl])
        nc.sync.dma_start(xiT8[:], xi8_d[:])
        for mt in range(MT):
            nc.sync.dma_start(xiT[mt][:], xi_d[mt * 128:(mt + 1) * 128, :])

        # --- small weights ---
        wrf = [ph0.tile([128, OUT_F], F32, name=f"wrf{mt}", tag="wrf", bufs=2) for mt in range(MT)]
        wr_sb = [consts.tile([128, OUT_F], BF16, name=f"wr{mt}", tag=f"wr{mt}") for mt in range(MT)]
        for mt in range(MT):
            nc.sync.dma_start(wrf[mt][:], wr_d[mt * 128:(mt + 1) * 128, :])
            nc.vector.tensor_copy(wr_sb[mt][:], wrf[mt][:])
        biasf = ph0.tile([1, OUT_F], F32, name="biasf", tag="biasf", bufs=1)
        nc.sync.dma_start(biasf[:], b_d.ap().rearrange("(a b) -> a b", a=1))
        bias_sb = consts.tile([1, OUT_F], BF16, name="bias", tag="bias")
        nc.vector.tensor_copy(bias_sb[:], biasf[:])
        wij_sb = []
        for h in range(HEADS):
            t = consts.tile([HF, 2], F32, name=f"wij{h}", tag=f"wij{h}")
            nc.sync.dma_start(t[:, 0:1], wi_d[h])
            nc.sync.dma_start(t[:, 1:2], wj_d[h])
            wij_sb.append(t)

        # rhsall8[p, kt, h*HC + c]: per-head [W_h(64) | A_h | B_h] moving
        # operand of the h matmul (fp8, k = kt*128 + p), A = W @ wi, B = W @ wj.
        rhsall8 = consts.tile([128, MT, HEADS * HC], F8, name="rhsall8", tag="rhsall8")
        wtmp = [consts.tile([128, HEADS * HF], F32, name=f"wtmp{mt}", tag=f"wtmp{mt}") for mt in range(MT)]
        for mt in range(MT):
            for h in range(HEADS):
                nc.sync.dma_start(wtmp[mt][:, h * HF:(h + 1) * HF],
                                  w_d[h, mt * 128:(mt + 1) * 128, :])
                nc.vector.tensor_copy(rhsall8[:, mt, h * HC:h * HC + HF],
                                      wtmp[mt][:, h * HF:(h + 1) * HF])

        # W_h^T (for A/B columns): transpose the [m, f] weight slices.
        whT = [consts.tile([HF, IN_F], F32, name=f"whT{h}", tag=f"whT{h}") for h in range(HEADS)]
        for h in range(HEADS):
            for mt in range(MT):
                ps = ph0ps.tile([HF, 128], F32, name="wtps", tag="wtps", bufs=1)
                nc.tensor.transpose(ps[:], wtmp[mt][:, h * HF:(h + 1) * HF], ident[:])
                nc.vector.tensor_copy(whT[h][:, mt * 128:(mt + 1) * 128], ps[:])
        for h in range(HEADS):
            for mt in range(MT):
                psab = ph0ps.tile([128, 2], F32, name="abps", tag="abps", bufs=1)
                nc.tensor.matmul(psab[:], whT[h][:, mt * 128:(mt + 1) * 128],
                                 wij_sb[h][:], start=True, stop=True)
                nc.vector.tensor_copy(rhsall8[:, mt, h * HC + HF:h * HC + HF + 2], psab[:])

        # --- r_i = exp(-0.8 ci) for our rows (early: unblocks main-loop lead) ---
        for h in range(HEADS):
            rrow = persist.tile([1, ROWS], F16, name=f"rrow{h}", tag=f"rrow{h}")
            for ch in range(ICH):
                psci = ph0ps.tile([2, 512], F32, name="psci", tag="psci", bufs=1)
                for mt in range(MT):
                    nc.tensor.matmul(psci[:], rhsall8[:, mt, h * HC + HF:h * HC + HF + 2],
                                     xiT8[:, mt, ch * 512:(ch + 1) * 512],
                                     start=(mt == 0), stop=(mt == MT - 1))
                nc.scalar.activation(rrow[0:1, ch * 512:(ch + 1) * 512],
                                     psci[0:1, :], AF.Exp, scale=-0.8)
            nc.gpsimd.partition_broadcast(Rb[h][:], rrow[:])

        # --- h / ci / cj for all N rows (fp8 DoubleRow: contraction 2x128) ---
        for jt in range(NJT):
            psh = ph0ps.tile([128, HEADS * HC], F32, name="psh", tag="psh", bufs=5)
            nc.tensor.matmul(psh[:], xT8[:, :, jt * 128:(jt + 1) * 128],
                             rhsall8[:], start=True, stop=True, perf_mode=DR)
            hdst = Hb[:, jt].rearrange("p a b -> p (a b)")
            if jt % 2 == 0:
                nc.scalar.copy(hdst, psh[:])
            else:
                nc.vector.tensor_copy(hdst, psh[:])

    # --- per-head j-row scalars from the cj column of Hb ---
    for h in range(HEADS):
        cjap = Hb[:, :, h, HF + 1]
        nc.scalar.activation(vs[h][:], cjap, AF.Exp)
        nc.scalar.activation(vps[h][:], cjap, AF.Exp, scale=0.2)
        # ones column for the softmax denominator (overwrites the unused slot)
        nc.scalar.activation(Hb[:, :, h, HF], cjap, AF.Copy, bias=1.0, scale=0.0)


    # --- main loop: scores + aggregation ---
    with tc.tile_pool(name="psy", bufs=HEADS * ICH, space="PSUM") as psy_pool:
        psy = [[psy_pool.tile([HF + 1, 512], F32, name="psy", tag="psy") for _ in range(ICH)]
               for _ in range(HEADS)]
        with tc.tile_pool(name="mainl", bufs=3) as ml:
            for jb in range(NJT):
                g8t = ml.tile([128, ROWS], F8, name="g8t", tag="g8t", bufs=3)
                nc.sync.dma_start(g8t[:], g8_d[jb * 128:(jb + 1) * 128, :])
                adj = ml.tile([128, ROWS], F16, name="adj", tag="adj", bufs=3)
                nc.scalar.copy(adj[:], g8t[:])
                wt4 = ml.tile([128, HEADS, ROWS], F16, name="wt4", tag="wt4", bufs=3)
                for h in range(HEADS):
                    ins = nc.vector._custom_dve(
                        _GSM_OP, out=wt4[:, h, :], in0=Rb[h][:], in1=adj[:],
                        s0=vps[h][:, jb:jb + 1], s1=vs[h][:, jb:jb + 1])
                    ins.ins.perf_max = 2
                last = jb == NJT - 1
                for h in range(HEADS):
                    for ch in range(ICH):
                        nc.tensor.matmul(psy[h][ch][:], Hb[:, jb, h, 0:HF + 1],
                                         wt4[:, h, ch * 512:(ch + 1) * 512],
                                         start=(jb == 0), stop=last)

        # copy numerators/denominator out of PSUM (releases psy banks)
        ysb = [[persist.tile([HF + 1, 512], F32, name=f"ysb{h}_{ch}", tag=f"ysb{h}_{ch}") for ch in range(ICH)]
               for h in range(HEADS)]
        for h in range(HEADS):
            for ch in range(ICH):
                if (h + ch) % 2 == 0:
                    nc.scalar.copy(ysb[h][ch][:], psy[h][ch][:])
                else:
                    nc.vector.tensor_copy(ysb[h][ch][:], psy[h][ch][:])

    # --- output: transpose to [i, f], divide by denominator, add residual ---
    with tc.tile_pool(name="outps", bufs=3, space="PSUM") as outps, \
         tc.tile_pool(name="outsb", bufs=3) as outsb:
        for it in range(ROWS // 128):
            ch, off = divmod(it * 128, 512)
            pso = outps.tile([128, HEADS, HF + 1], F32, name="pso", tag="pso")
            for h in range(HEADS):
                nc.tensor.transpose(pso[:, h, :], ysb[h][ch][:, off:off + 128],
                                    ident[0:HF + 1, 0:HF + 1])
            rden = outsb.tile([128, HEADS], F32, name="rden", tag="rden")
            nc.vector.reciprocal(rden[:], pso[:, :, HF])
            yatt = outsb.tile([128, OUT_F], F32, name="yatt", tag="yatt")
            for h in range(HEADS):
                nc.scalar.activation(yatt[:, h * HF:(h + 1) * HF], pso[:, h, 0:HF],
                                     AF.Copy, scale=rden[:, h:h + 1])
            psr = outps.tile([128, OUT_F], F32, name="psr", tag="psr")
            for mt in range(MT):
                nc.tensor.matmul(psr[:], xiT[mt][:, it * 128:(it + 1) * 128],
                                 wr_sb[mt][:], start=(mt == 0), stop=False)
            nc.tensor.matmul(psr[:], ones1[:], bias_sb[:], start=False, stop=True)
            out_t = outsb.tile([128, OUT_F], F32, name="outt", tag="outt")
            nc.vector.tensor_tensor(out_t[:], yatt[:], psr[:], ALU.add)
            nc.sync.dma_start(y_d[it * 128:(it + 1) * 128, :], out_t[:])


_NC_CACHE = {}


def _get_program(loop_reps=None):
    if loop_reps not in _NC_CACHE:
        _NC_CACHE[loop_reps] = _build_program(loop_reps)
    return _NC_CACHE[loop_reps]


def _make_in_maps(x, graph, weight, weight_i, weight_j, weight_r, bias):
    import ml_dtypes
    f8 = mybir.dt.np(F8)
    x = np.asarray(x, dtype=np.float32)
    xt = np.ascontiguousarray(x.T)                      # [IN_F, N] f32
    xt8 = np.ascontiguousarray(                         # [128, MT, N] fp8
        xt.reshape(MT, 128, N).transpose(1, 0, 2).astype(f8))
    xt_bf = xt.astype(ml_dtypes.bfloat16)
    g8 = np.asarray(graph, dtype=np.float32).astype(f8)
    maps = []
    for c in range(NCORES):
        i0 = c * ROWS
        maps.append({
            "xt8": xt8,
            "xit8": np.ascontiguousarray(xt8[:, :, i0:i0 + ROWS]),
            "xit": np.ascontiguousarray(xt_bf[:, i0:i0 + ROWS]),
            "gcol8": np.ascontiguousarray(g8[:, i0:i0 + ROWS]),
            "weight": np.ascontiguousarray(weight, dtype=np.float32),
            "weight_i": np.ascontiguousarray(weight_i, dtype=np.float32),
            "weight_j": np.ascontiguousarray(weight_j, dtype=np.float32),
            "weight_r": np.ascontiguousarray(weight_r, dtype=np.float32),
            "bias": np.ascontiguousarray(bias, dtype=np.float32),
            "ident": np.eye(128, dtype=np.float32),
        })
    return maps


def _run(in_maps, loop_reps=None):
    nc = _get_program(loop_reps)
    res = run_bass_kernel_spmd(nc, in_maps, list(range(NCORES)))
    return np.concatenate([res.results[c]["y"] for c in range(NCORES)], axis=0)


def kernel(x, graph, weight, weight_i, weight_j, weight_r, bias):
    in_maps = _make_in_maps(x, graph, weight, weight_i, weight_j, weight_r, bias)
    return _run(in_maps).astype(np.float32)
